# revision 50
# baseline (speedup 1.0000x reference)
"""Trainium2 Bass kernel for nn_AttentionMLP (pairwise-MLP attention + softmax).

Math (per batch b):
  hA = inputA[b] @ W1[:128]          # (K, H)
  hB = inputB[b] @ W1[128:]          # (L, H)
  scores[k, l] = sum_h relu(hA[k, h] + hB[l, h] + b1[h]) * w2[h]
  out[b, k, :] = softmax(scores[k, :])

Shapes: B=2, K=128, L=4096, D=H=128.

Distribution: pure data parallel over the (b, k) grid — core c handles
b = c // 4 and a 32-wide k block (no collectives; the softmax over L is
core-local).

Per-core device algorithm (SBUF partition axis = H), v2:
  1. hBT = W1b.T @ inputB[b].T -> PSUM, copied to SBUF as bf16 [128, 4096]
     (inputs pre-transposed on host so the contraction dim lands on
     partitions). Input DMA fanned out over the SP and ACT HWDGE queues.
  2. bias[:, k] = W1a.T @ inputA[b].T + b1  (fp32 [128, 32])
  3. Per k: R_k = relu(hBT + bias[:, k]) as one [128, 4096] bf16 pass —
     VectorE tensor_scalar (add+max, 4x mode) for 24 k's, ScalarE
     activation (per-partition bias) for 8, statically load-balanced.
     (GpSimd/Pool tensor_scalar was tried and measured ~50x slower than
     the cost model predicts — ~57us per pass — so Pool gets none.)
  4. scores = w2.T @ R_k via M=32 matmuls (N=512 chunks) whose weight
     matrix is a 32-wide slice of a zeros|w2|zeros band — w2 lands in
     column 4*(k%8)+(chunk//2), so chunk c of k accumulates into PSUM
     partition 4*k + c//2, columns 512*(c%2):...  All 256 matmuls
     accumulate into ONE [128, 1024] PSUM tile (2 banks) holding the
     scores in softmax layout: partition 4k+q = l-range [1024q:1024(q+1)).
     Four matmuls run concurrently via PE col-tiling (tile_position=
     (0,32j), k's 8 apart).
  5. Softmax without max-subtraction (scores are O(1)): ScalarE exp reads
     the PSUM tile directly (this is also the PSUM evacuation) with
     accum_out producing row sums; per-k sums = quarter sums combined and
     broadcast back via tiny 0/1 matmuls; final scale on VectorE; output
     DMA split over the SP and ACT queues.

All per-iteration tiles live in bufs=2 pools and the hardware loop runs
four body copies per For_i iteration in H H T H T H T T order, so
consecutive bodies double-buffer (body i+1's input DMA + hBT production
overlap body i's relu/score work) and each softmax tail issues behind a
later body's relu passes — the exposed end-of-iteration tail is paid
once per four bodies.
"""

import os
import sys

for _p in ("/opt/trn_rl_repo", "/root/.axon_site/_ro/trn_rl_repo"):
    if os.path.isdir(_p) and _p not in sys.path:
        sys.path.insert(0, _p)

import numpy as np
import ml_dtypes

BF = ml_dtypes.bfloat16
B, K, L, D, H = 2, 128, 4096, 128, 128
NCORES = 8
KPC = 32   # k's per core
NG = 8     # concurrency groups; group g = k's {g, 8+g, 16+g, 24+g}

import json as _json


def _env_pairs(name, default):
    v = os.environ.get(name)
    return {tuple(p) for p in _json.loads(v)} if v else set(default)


def _env_set(name, default):
    v = os.environ.get(name)
    return set(_json.loads(v)) if v else set(default)


# relu passes on ScalarE / GpSimd (rest on VectorE).  GpSimd's
# tensor_scalar measured ~50x slower than the cost model predicts
# (~57us per [128,4096] pass), so POOL_PASSES defaults to empty.
ACT_PASSES = _env_pairs("KERNEL_ACT_PASSES",
                        [(0, 3), (1, 1), (2, 2), (3, 3), (4, 0), (5, 1),
                         (6, 2), (7, 3)])
POOL_PASSES = _env_pairs("KERNEL_POOL_PASSES", [])
# hbt chunk copies on ScalarE (rest on VectorE)
ACT_COPIES = _env_set("KERNEL_ACT_COPIES", [1, 5])
# ablation variant: base | peonly | reluonly | skeleton
VARIANT = os.environ.get("KERNEL_VARIANT", "base")
# fan input/output DMAs across SP+ACT queues (else all SP)
DMA_SPLIT = os.environ.get("KERNEL_DMA_SPLIT", "1") == "1"
# loop body order: 1 = H(0) H(1) T(0) T(1), 0 = H(0) T(0) H(1) T(1).
# H H T T measures ~1.7us faster: body i's exp sits behind body i+1's
# relu passes in the ScalarE queue, so the engine is never parked
# waiting for body i's PE drain.
PIPE_TAILS = os.environ.get("KERNEL_PIPE_TAILS", "1") == "1"
# second DMA queue: act (HWDGE) or pool (SWDGE)
DMA_QUEUE2 = os.environ.get("KERNEL_DMA_QUEUE2", "act")
# number of input DMA chunks (split round-robin over the queues)
IN_CHUNKS = int(os.environ.get("KERNEL_IN_CHUNKS", "4"))
# softmax-denominator fold: dve (stream_shuffle) or pe (0/1-band matmuls).
# Measured same-window A/B: pe 42.2us vs dve 43.7us — the 7-op serial
# DVE fold extends VectorE's critical chain, while the PE is idle at
# tail time, so the cross-engine matmul version wins.
TAIL_FOLD = os.environ.get("KERNEL_TAIL_FOLD", "pe")
# single [128,1024] exp + mult instead of two 512-wide halves (the
# half-split's PSUM-bank overlap trick is redundant under H H T T order;
# merging drops one ScalarE SBUF bubble, one VectorE op and one matmul)
MERGED_TAIL = os.environ.get("KERNEL_MERGED_TAIL", "1") == "1"
# split group-0 relu passes in column halves (earlier PE start per body
# at the cost of extra per-instruction overhead)
G0_HALVES = os.environ.get("KERNEL_G0_HALVES", "1") == "1"
# 4-body pipeline per loop iteration (H H T H T H T T): end-of-iteration
# tail exposure paid once per 4 bodies instead of per 2
PIPE4 = os.environ.get("KERNEL_PIPE4", "1") == "1"
# pipeline depth for the loop body (H H (T H)xN T T); 4 measured 38.7us
# vs 42.0 at depth 2
PIPE_DEPTH = int(os.environ.get("KERNEL_PIPE_DEPTH", "4"))
# second output DMA on queue 2 (else both on SP)
OUT_SPLIT = os.environ.get("KERNEL_OUT_SPLIT", "1") == "1"

_BUILT = None


def _build(reps=1, loop=False, act_passes=None, pool_passes=None,
           act_copies=None):
    global ACT_PASSES, POOL_PASSES, ACT_COPIES
    if act_passes is not None:
        ACT_PASSES = {tuple(p) for p in act_passes}
    if pool_passes is not None:
        POOL_PASSES = {tuple(p) for p in pool_passes}
    if act_copies is not None:
        ACT_COPIES = set(act_copies)
    import concourse.mybir as mybir
    import concourse.tile as tile
    from concourse import bacc

    dt = mybir.dt
    f32, bf = dt.float32, dt.bfloat16
    AF = mybir.ActivationFunctionType
    ALU = mybir.AluOpType

    nc = bacc.Bacc("TRN2", target_bir_lowering=False, debug=False,
                   enable_asserts=True)

    xbt = nc.dram_tensor("xbt", [128, L], bf, kind="ExternalInput").ap()
    xat = nc.dram_tensor("xat", [128, KPC], bf, kind="ExternalInput").ap()
    w1a = nc.dram_tensor("w1a", [128, H], bf, kind="ExternalInput").ap()
    w1b = nc.dram_tensor("w1b", [128, H], bf, kind="ExternalInput").ap()
    b1c = nc.dram_tensor("b1c", [128, 1], f32, kind="ExternalInput").ap()
    wband = nc.dram_tensor("wband", [128, 64], bf, kind="ExternalInput").ap()
    wcomb = nc.dram_tensor("wcomb", [128, KPC], f32, kind="ExternalInput").ap()
    wbcast = nc.dram_tensor("wbcast", [KPC, 128], f32, kind="ExternalInput").ap()
    out = nc.dram_tensor("out", [128, 1024], f32, kind="ExternalOutput").ap()

    with tile.TileContext(nc) as tc:
        with (
            tc.tile_pool(name="consts", bufs=1) as consts,
            tc.tile_pool(name="work", bufs=2) as work,
            tc.tile_pool(name="rpool", bufs=int(os.environ.get("KERNEL_RBUFS", "12"))) as rpool,
            tc.tile_pool(name="psum", bufs=4, space="PSUM") as psum,
            tc.tile_pool(name="epsum", bufs=2, space="PSUM") as epsum,
        ):
            w1a_sb = consts.tile([128, H], bf, tag="w1a")
            nc.sync.dma_start(w1a_sb[:], w1a)
            w1b_sb = consts.tile([128, H], bf, tag="w1b")
            nc.sync.dma_start(w1b_sb[:], w1b)
            xat_sb = consts.tile([128, KPC], bf, tag="xat")
            nc.sync.dma_start(xat_sb[:], xat)
            b1_sb = consts.tile([128, 1], f32, tag="b1")
            nc.sync.dma_start(b1_sb[:], b1c)
            wband_sb = consts.tile([128, 64], bf, tag="wband")
            nc.sync.dma_start(wband_sb[:], wband)
            wcomb_sb = consts.tile([128, KPC], f32, tag="wcomb")
            nc.sync.dma_start(wcomb_sb[:], wcomb)
            wbcast_sb = consts.tile([KPC, 128], f32, tag="wbcast")
            nc.sync.dma_start(wbcast_sb[:], wbcast)
            # dummy ACT op issued first so the ~2.7us activation-table load
            # overlaps the input DMAs instead of stalling the first real
            # ScalarE op.  Exp anchors the exp_and_others table set, which
            # also holds Relu and Copy — one table load serves the kernel.
            warm_sb = consts.tile([128, 1], f32, tag="warm")
            nc.vector.memset(warm_sb[:], 0.0)
            nc.scalar.activation(warm_sb[:], warm_sb[:], AF.Exp)

            args = (nc, work, rpool, psum, epsum, xbt, out, w1a_sb, w1b_sb,
                    xat_sb, b1_sb, wband_sb, wcomb_sb, wbcast_sb, f32, bf,
                    AF, ALU)
            if loop and reps > 1:
                # software-pipelined: heads issue before earlier bodies'
                # tails, so each engine's queue stays busy with later
                # bodies' relu while earlier bodies' PE drains to the exp
                if PIPE4 and reps % PIPE_DEPTH == 0:
                    with tc.For_i(0, reps // PIPE_DEPTH, 1):
                        ctxs = [_head(*args), _head(*args)]
                        for i in range(PIPE_DEPTH - 2):
                            _tail(ctxs[i])
                            ctxs.append(_head(*args))
                        _tail(ctxs[-2])
                        _tail(ctxs[-1])
                else:
                    assert reps % 2 == 0
                    with tc.For_i(0, reps // 2, 1):
                        if PIPE_TAILS:
                            c0 = _head(*args)
                            c1 = _head(*args)
                            _tail(c0)
                            _tail(c1)
                        else:
                            _tail(_head(*args))
                            _tail(_head(*args))
            else:
                for _rep in range(reps):
                    _tail(_head(*args))

    nc.compile()
    return nc


def _head(nc, work, rpool, psum, epsum, xbt, out, w1a_sb, w1b_sb, xat_sb,
          b1_sb, wband_sb, wcomb_sb, wbcast_sb, f32, bf, AF, ALU):
    # --- head: input DMA (4 chunks over 2 HWDGE queues) + hBT + bias ---
    eng2 = nc.gpsimd if DMA_QUEUE2 == "pool" else nc.scalar
    xbt_sb = work.tile([128, L], bf, tag="xbt")
    cw = L // IN_CHUNKS
    for c in range(IN_CHUNKS):
        eng = eng2 if (DMA_SPLIT and c % 2 == 1) else nc.sync
        eng.dma_start(xbt_sb[:, cw * c:cw * (c + 1)],
                      xbt[:, cw * c:cw * (c + 1)])

    ps_h = psum.tile([128, 512], f32, tag="ps")
    nc.tensor.matmul(ps_h[:, 0:KPC], lhsT=w1a_sb[:], rhs=xat_sb[:],
                     start=True, stop=True)
    bias_sb = work.tile([128, KPC], f32, tag="bias")
    nc.vector.tensor_scalar(out=bias_sb[:], in0=ps_h[:, 0:KPC],
                            scalar1=b1_sb[:, 0:1], scalar2=None,
                            op0=ALU.add)

    hbt_sb = work.tile([128, L], bf, tag="hbt")
    for c in range(8):
        ps_c = psum.tile([128, 512], f32, tag="ps")
        sl = slice(512 * c, 512 * c + 512)
        nc.tensor.matmul(ps_c[:], lhsT=w1b_sb[:], rhs=xbt_sb[:, sl],
                         start=True, stop=True)
        if c in ACT_COPIES:
            nc.scalar.copy(hbt_sb[:, sl], ps_c[:])
        else:
            nc.vector.tensor_copy(hbt_sb[:, sl], ps_c[:])

    # --- scores: relu passes (3 engines) + banded matmuls into PSUM ---
    # partition 4k+q holds l-range [1024q : 1024(q+1))
    e_ps = epsum.tile([128, 1024], f32, tag="eps")

    pe_ablate = VARIANT in ("peonly", "skeleton")
    for g in range(NG):
        if VARIANT == "skeleton":
            break
        rts = []
        for j in range(4):
            k = 8 * j + g
            if pe_ablate:
                rts.append(hbt_sb)
                continue
            rt = rpool.tile([128, L], bf, tag="r")
            # group 0 passes split in halves: the first half only needs
            # hbt chunks 0-3, so R production (and the PE) starts earlier
            # in the single-shot execution
            halves = ((0, 2048), (2048, 4096)) if (g == 0 and G0_HALVES) \
                else ((0, 4096),)
            for lo, hi in halves:
                if (g, j) in ACT_PASSES:
                    nc.scalar.activation(rt[:, lo:hi], hbt_sb[:, lo:hi],
                                         AF.Relu,
                                         bias=bias_sb[:, k:k + 1],
                                         scale=1.0)
                elif (g, j) in POOL_PASSES:
                    nc.gpsimd.tensor_scalar(
                        out=rt[:, lo:hi], in0=hbt_sb[:, lo:hi],
                        scalar1=bias_sb[:, k:k + 1], scalar2=0.0,
                        op0=ALU.add, op1=ALU.max)
                else:
                    nc.vector.tensor_scalar(
                        out=rt[:, lo:hi], in0=hbt_sb[:, lo:hi],
                        scalar1=bias_sb[:, k:k + 1], scalar2=0.0,
                        op0=ALU.add, op1=ALU.max)
            rts.append(rt)
        if VARIANT == "reluonly":
            continue
        # q-major: one weight slice serves 8 matmuls; in the last group
        # run all win=0 matmuls first so the exp on the first PSUM bank
        # can overlap the win=1 matmuls
        if g < NG - 1:
            order = [(2 * q + win, j) for q in range(4)
                     for j in range(4) for win in range(2)]
        else:
            order = ([(2 * q, j) for q in range(4) for j in range(4)]
                     + [(2 * q + 1, j) for q in range(4) for j in range(4)])
        for c, j in order:
            win = c % 2
            ncol = 256 if VARIANT == "fullhalfpe" else 512
            v = 4 * g + c // 2  # local column for w2
            nc.tensor.matmul(
                e_ps[32 * j:32 * j + 32, 512 * win:512 * win + ncol],
                lhsT=wband_sb[:, 31 - v:63 - v],
                rhs=rts[j][:, 512 * c:512 * c + ncol],
                start=(g == 0 and c // 2 == 0),
                stop=(g == NG - 1 and c // 2 == 3),
                tile_position=(0, 32 * j),
                skip_group_check=True)
    if VARIANT in ("reluonly", "skeleton"):
        nc.vector.memset(e_ps[:, 0:1024], 0.0)

    return (nc, work, psum, out, wcomb_sb, wbcast_sb, f32, AF, e_ps)


def _tail(ctx):
    nc, work, psum, out, wcomb_sb, wbcast_sb, f32, AF, e_ps = ctx

    # --- softmax tail; exp directly off PSUM = evacuation ---
    e2_sb = work.tile([128, 1024], f32, tag="exp")
    s0_sb = work.tile([128, 1], f32, tag="sums0")
    s1_sb = work.tile([128, 1], f32, tag="sums1")
    if MERGED_TAIL:
        nc.scalar.activation(e2_sb[:, 0:1024], e_ps[:, 0:1024], AF.Exp,
                             accum_out=s0_sb[:, 0:1])
        sums = [s0_sb]
    else:
        nc.scalar.activation(e2_sb[:, 0:512], e_ps[:, 0:512], AF.Exp,
                             accum_out=s0_sb[:, 0:1])
        nc.scalar.activation(e2_sb[:, 512:1024], e_ps[:, 512:1024], AF.Exp,
                             accum_out=s1_sb[:, 0:1])
        sums = [s0_sb, s1_sb]
    if TAIL_FOLD == "dve":
        # per-k denominator = sum of the 4 quarter-row sums (partitions
        # 4k..4k+3) folded entirely on VectorE: two rotate-within-4
        # stream_shuffles.  No PE matmuls or PSUM round-trips in the tail.
        rot1 = [(m & ~3) | ((m + 1) & 3) for m in range(32)]
        rot2 = [(m & ~3) | ((m + 2) & 3) for m in range(32)]
        if len(sums) == 2:
            ss_sb = work.tile([128, 1], f32, tag="ssum")
            nc.vector.tensor_add(out=ss_sb[:], in0=s0_sb[:, 0:1],
                                 in1=s1_sb[:, 0:1])
        else:
            ss_sb = s0_sb
        sh1_sb = work.tile([128, 1], f32, tag="sh1")
        nc.vector.stream_shuffle(sh1_sb[:], ss_sb[:], rot1)
        u1_sb = work.tile([128, 1], f32, tag="u1")
        nc.vector.tensor_add(out=u1_sb[:], in0=ss_sb[:], in1=sh1_sb[:])
        sh2_sb = work.tile([128, 1], f32, tag="sh2")
        nc.vector.stream_shuffle(sh2_sb[:], u1_sb[:], rot2)
        dn_sb = work.tile([128, 1], f32, tag="denom")
        nc.vector.tensor_add(out=dn_sb[:], in0=u1_sb[:], in1=sh2_sb[:])
        tr_sb = work.tile([128, 1], f32, tag="recip")
        nc.vector.reciprocal(tr_sb[:], dn_sb[:])
        scal = tr_sb[:, 0:1]
    else:
        # combine the quarter-row sums via 0/1-band PE matmuls + PSUM
        ps_t = psum.tile([128, 512], f32, tag="ps")
        for i, s in enumerate(sums):
            nc.tensor.matmul(ps_t[0:KPC, 0:1], lhsT=wcomb_sb[:],
                             rhs=s[:, 0:1], start=(i == 0),
                             stop=(i == len(sums) - 1))
        tq_sb = work.tile([KPC, 1], f32, tag="recipq")
        nc.vector.reciprocal(tq_sb[:], ps_t[0:KPC, 0:1])
        ps_u = psum.tile([128, 512], f32, tag="ps")
        nc.tensor.matmul(ps_u[:, 0:1], lhsT=wbcast_sb[:], rhs=tq_sb[:],
                         start=True, stop=True)
        scal = ps_u[:, 0:1]
    f_sb = work.tile([128, 1024], f32, tag="final")
    if MERGED_TAIL:
        nc.vector.tensor_scalar_mul(out=f_sb[:, 0:1024],
                                    in0=e2_sb[:, 0:1024], scalar1=scal)
        nc.sync.dma_start(out[:, 0:512], f_sb[:, 0:512])
    else:
        nc.vector.tensor_scalar_mul(out=f_sb[:, 0:512], in0=e2_sb[:, 0:512],
                                    scalar1=scal)
        nc.sync.dma_start(out[:, 0:512], f_sb[:, 0:512])
        nc.vector.tensor_scalar_mul(out=f_sb[:, 512:1024],
                                    in0=e2_sb[:, 512:1024],
                                    scalar1=scal)
    eng2 = nc.gpsimd if DMA_QUEUE2 == "pool" else nc.scalar
    (eng2 if (DMA_SPLIT and OUT_SPLIT) else nc.sync).dma_start(
        out[:, 512:1024], f_sb[:, 512:1024])


def _get_built():
    global _BUILT
    if _BUILT is None:
        _BUILT = _build()
    return _BUILT


def make_in_maps(inputA, inputB, W1, b1, w2):
    wband = np.zeros((128, 64), np.float32)
    wband[:, 31] = w2
    wcomb = (np.arange(128)[:, None] // 4 == np.arange(KPC)[None, :]) \
        .astype(np.float32)
    wbcast = (np.arange(128)[None, :] // 4 == np.arange(KPC)[:, None]) \
        .astype(np.float32)
    w1a = np.ascontiguousarray(W1[:D]).astype(BF)
    w1b = np.ascontiguousarray(W1[D:]).astype(BF)
    b1c = np.ascontiguousarray(b1.reshape(128, 1)).astype(np.float32)
    wband = wband.astype(BF)
    in_maps = []
    for core in range(NCORES):
        b, kq = core // 4, core % 4
        k0 = KPC * kq
        in_maps.append({
            "xbt": np.ascontiguousarray(inputB[b].T).astype(BF),
            "xat": np.ascontiguousarray(inputA[b, k0:k0 + KPC].T).astype(BF),
            "w1a": w1a, "w1b": w1b, "b1c": b1c, "wband": wband,
            "wcomb": wcomb, "wbcast": wbcast,
        })
    return in_maps


def assemble(results):
    """results: list of 8 dicts with 'out' [128, 1024] f32."""
    full = np.empty((B, K, L), np.float32)
    for core in range(NCORES):
        b, kq = core // 4, core % 4
        full[b, KPC * kq:KPC * (kq + 1)] = \
            np.asarray(results[core]["out"]).reshape(KPC, L)
    return full


def kernel(**inputs):
    from concourse.bass_utils import run_bass_kernel_spmd

    inputA = np.asarray(inputs["inputA"], np.float32)
    inputB = np.asarray(inputs["inputB"], np.float32)
    W1 = np.asarray(inputs["W1"], np.float32)
    b1 = np.asarray(inputs["b1"], np.float32)
    w2 = np.asarray(inputs["w2"], np.float32)

    nc = _get_built()
    in_maps = make_in_maps(inputA, inputB, W1, b1, w2)
    res = run_bass_kernel_spmd(nc, in_maps, core_ids=list(range(NCORES)))
    return assemble(res.results)
